# revision 61
# baseline (speedup 1.0000x reference)
"""Multi-head causal attention (B=2, T=2048, C=1024, H=16, D=64) on 8 trn2 cores.

Sharding: core c -> batch b = c//4, head group g = c%4 (4 heads each),
Megatron-style: QKV column-parallel, proj row-parallel. Partial outputs are
summed on the host; bk is softmax-invariant and dropped, bv/bp fold into a
host-side constant. All matmul operands are bf16 (host-cast), accumulation
and softmax stay fp32.

Device kernel (per core):
  All inputs are host-packed into a handful of big partition-major tiles so
  the whole load is 8 dma_starts (each costs ~680ns serialized on the sync
  engine - per-slice loads would gate the kernel on sync for ~50us).
  Q^T (+bq) / K^T = W.T @ A        [256, 2048]   channels on partitions
  V = A.T @ Wv_loc.T               [2048, 4*(64+1)]  natural layout, a ones
                                   column per head for softmax denominators
  Attention runs on HEAD PAIRS (heads 2hp, 2hp+1 live on partition halves
  0:64 / 64:128 of the qt/kt tiles):
     S^T even/odd = K_h^T.T @ Q_h^T   two K=64 matmuls into adjacent PSUM
                                      banks; disjoint PE row-groups (0,0) /
                                      (64,0) -> they run CONCURRENTLY
     P^T pair = exp(0.125*S^T)        ONE ACT call over both banks (N=2*w)
                                      amortizing the ~352-cycle ACT overhead
     diagonal chunks masked on GpSimd via one paired affine_select
     PV~[65, 2*512] += V~_h.T @ P^T   row 64 accumulates the denominator l
     out^T = PV[0:64] * bcast(1/l)    approx-recip + paired broadcast,
                                      multiplied straight out of PSUM
  The QKV/V projection chains are SPREAD one-at-a-time through the attention
  stream, so the PE never idles (and its HAM clock stays warm) while the
  scalar engine grinds exps.
  Y = attn-out^T.T @ Wp_loc.T      [2048, 1024]  bf16 partial, proj
                                   interleaved per q-chunk, summed on host
"""

import sys

sys.path.insert(0, "/opt/trn_rl_repo")

import numpy as np
import ml_dtypes

NP_DT = ml_dtypes.bfloat16

import concourse.bass as bass  # noqa: F401
import concourse.mybir as mybir
import concourse.tile as tile
from concourse import bacc
from concourse.bass_utils import run_bass_kernel_spmd

N_CORES = 8
B, T, C = 2, 2048, 1024
H, D = 16, 64
H_LOC = 4              # heads per core
OL = H_LOC * D         # local channels = 256
CQ = 512               # PSUM-bank q chunk
CK = 128               # k chunk (partition dim)
NT = T // 128          # 16
KC = C // 128          # 8 contraction chunks for QKV

f32 = mybir.dt.float32
bf16 = mybir.dt.bfloat16
DT = bf16  # matmul operand dtype

_COMPILED = None


def _build():
    nc = bacc.Bacc("TRN2", debug=False, num_devices=N_CORES)

    # host-packed partition-major inputs (see make_in_maps)
    AB_d = nc.dram_tensor("AB", [128, 4 * KC * CQ], DT, kind="ExternalInput").ap()
    WB_d = nc.dram_tensor("WB", [128, KC * CQ], DT, kind="ExternalInput").ap()
    WVB_d = nc.dram_tensor("WVB", [128, KC * OL], DT, kind="ExternalInput").ap()
    WPB_d = nc.dram_tensor("WPB", [128, 2 * C], DT, kind="ExternalInput").ap()
    BQ_d = nc.dram_tensor("BQ", [128, 2], f32, kind="ExternalInput").ap()
    Y = nc.dram_tensor("Y", [T, C], DT, kind="ExternalOutput").ap()

    Exp = mybir.ActivationFunctionType.Exp

    with tile.TileContext(nc) as tc:
        with tc.tile_pool(name="sbuf", bufs=1) as pool, \
             tc.tile_pool(name="work", bufs=1) as wpool, \
             tc.tile_pool(name="psum", bufs=1, space="PSUM") as psum:

            # ---- resident inputs: 8 dma_starts in consumption order ----
            bq2 = pool.tile([128, 2], f32, tag="BQ", name="bq2")
            nc.sync.dma_start(bq2[:], BQ_d[:, :])
            # wB/aB0 land in kc-quarters, interleaved, so (a) the first QK
            # chain's early kc matmuls start before the full tiles arrive
            # and (b) more DMA queues engage while the engines ramp up
            QB = KC * CQ // 4
            wB = pool.tile([128, KC * CQ], DT, tag="WB", name="wB")
            aB = [pool.tile([128, KC * CQ], DT, tag=f"AB{n}", name=f"aB{n}")
                  for n in range(4)]
            for q in range(4):
                nc.sync.dma_start(wB[:, q * QB:(q + 1) * QB],
                                  WB_d[:, q * QB:(q + 1) * QB])
                nc.sync.dma_start(aB[0][:, q * QB:(q + 1) * QB],
                                  AB_d[:, q * QB:(q + 1) * QB])
            wvB = pool.tile([128, KC * OL], DT, tag="WVB", name="wvB")
            nc.sync.dma_start(wvB[:], WVB_d[:, :])
            for n in range(1, 4):
                nc.sync.dma_start(aB[n][:],
                                  AB_d[:, n * KC * CQ:(n + 1) * KC * CQ])
            wpB = pool.tile([128, 2 * C], DT, tag="WPB", name="wpB")
            nc.sync.dma_start(wpB[:], WPB_d[:, :])

            # ---- persistent intermediates (qt/kt per 512-col chunk so the
            # interleaved attention never waits on unrelated chunk writes)
            qt_sb = [[pool.tile([128, CQ], DT, tag=f"QT{i}_{n}",
                                name=f"qt{i}_{n}") for n in range(4)]
                     for i in range(2)]
            kt_sb = [[pool.tile([128, CQ], DT, tag=f"KT{i}_{n}",
                                name=f"kt{i}_{n}") for n in range(4)]
                     for i in range(2)]
            v_sb = [pool.tile([128, H_LOC * (D + 1)], DT, tag=f"V{i}",
                              name=f"v{i}") for i in range(NT)]
            ao_sb = [pool.tile([128, T], DT, tag=f"AO{i}", name=f"ao{i}")
                     for i in range(2)]

            # ones columns for the softmax denominators: one strided memset
            # per V tile, done up front off the critical path
            for tt in range(NT):
                nc.vector.memset(
                    v_sb[tt].rearrange("p (h x) -> p h x", x=D + 1)[:, :, D:D + 1],
                    1.0)
            # warm the GpSimd ucode paths and preload the ACT exp table so
            # neither cold-start lands mid-attention
            warm = wpool.tile([128, 8], f32, tag="warm")
            nc.vector.memset(warm[:], 1.0)
            nc.gpsimd.affine_select(
                out=warm[:], in_=warm[:],
                compare_op=mybir.AluOpType.is_ge, fill=0.0, base=0,
                pattern=[[1, 8]], channel_multiplier=-1)
            warm2 = wpool.tile([128, 8], f32, tag="warm2")
            nc.gpsimd.partition_broadcast(warm2[:], warm[0:1, :])
            warm3 = wpool.tile([128, 8], f32, tag="warm3")
            nc.scalar.activation(warm3[:], warm2[:], Exp, scale=0.001)
            # dummy matmuls during the ~15us input-DMA window: ~5us of PE
            # activity flips the HAM clock gate to 8/8 so the first QKV
            # chains run at 2.4GHz instead of 1.2
            wmm = wpool.tile([128, CQ], DT, tag="wmm")
            nc.vector.memset(wmm[:], 0.0)
            for _ in range(5):
                pw = psum.tile([128, CQ], f32, tag="prj", bufs=2, name="pw")
                for k in range(8):
                    nc.tensor.matmul(pw[:, 0:128], wmm[:, 0:128],
                                     wmm[:, 0:128],
                                     start=(k == 0), stop=(k == 7))

            # ---- QKV / V projection chain emitters (one chain each) ----
            def emit_qk_chain(n, m):
                ps = psum.tile([128, CQ], f32, tag="prj", bufs=2, name="ps")
                for kc in range(KC):
                    nc.tensor.matmul(
                        ps[:],
                        wB[:, kc * CQ + m * 128:kc * CQ + (m + 1) * 128],
                        aB[n][:, kc * CQ:(kc + 1) * CQ],
                        start=(kc == 0), stop=(kc == KC - 1))
                if m < 2:
                    nc.vector.tensor_scalar_add(
                        qt_sb[m][n][:], ps[:], bq2[:, m:m + 1])
                else:
                    nc.vector.tensor_copy(kt_sb[m - 2][n][:], ps[:])

            def emit_v_chain(tt):
                ps = psum.tile([128, CQ], f32, tag="prj", bufs=2,
                               name="psv")[:, 0:OL]
                for kc in range(KC):
                    nc.tensor.matmul(
                        ps[:],
                        aB[tt // 4][:, kc * CQ + (tt % 4) * 128:
                                    kc * CQ + (tt % 4 + 1) * 128],
                        wvB[:, kc * OL:(kc + 1) * OL],
                        start=(kc == 0), stop=(kc == KC - 1))
                nc.vector.tensor_copy(
                    v_sb[tt].rearrange("p (h x) -> p h x", x=D + 1)[:, :, 0:D],
                    ps.rearrange("p (h x) -> p h x", x=D))

            yt_tiles = {}
            tailw = {}

            def emit_proj_chain(tt, n2):
                if n2 == 0:
                    yt_tiles[tt] = wpool.tile([128, C], DT, tag="y", bufs=3,
                                              name="yt")
                yt = yt_tiles[tt]
                if tt >= 12:
                    # final q chunk: the attention "sp" PSUM banks are free
                    # in the tail, so pack 2 chains per [128,1024] sp tile
                    # (4 chains in flight vs prj's 2) — the 8-chain burst
                    # runs back-to-back with evictions trailing on both
                    # ACT and DVE, and the HAM clock never drops
                    c = (tt - 12) * 2 + n2
                    if c % 2 == 0:
                        tailw[c // 2] = psum.tile([128, 2 * CQ], f32,
                                                  tag="sp", bufs=2,
                                                  name="pst")
                    ps = tailw[c // 2][:, (c % 2) * CQ:(c % 2 + 1) * CQ]
                else:
                    ps = psum.tile([128, CQ], f32, tag="prj", bufs=2,
                                   name="psp")
                for kc2 in range(2):
                    nc.tensor.matmul(
                        ps[:],
                        ao_sb[kc2][:, tt * 128:(tt + 1) * 128],
                        wpB[:, kc2 * C + n2 * CQ:kc2 * C + (n2 + 1) * CQ],
                        start=(kc2 == 0), stop=(kc2 == 1))
                if tt >= 12:
                    if n2 == 0:
                        nc.scalar.copy(yt[:, 0:CQ], ps[:])
                    else:
                        nc.vector.tensor_copy(yt[:, CQ:2 * CQ], ps[:])
                    nc.sync.dma_start(
                        Y[tt * 128:(tt + 1) * 128, n2 * CQ:(n2 + 1) * CQ],
                        yt[:, n2 * CQ:(n2 + 1) * CQ])
                    if n2 == 1:
                        del yt_tiles[tt]
                    return
                nc.vector.tensor_copy(yt[:, n2 * CQ:(n2 + 1) * CQ], ps[:])
                if n2 == 1:
                    del yt_tiles[tt]
                    nc.sync.dma_start(Y[tt * 128:(tt + 1) * 128, :], yt[:])

            # ---- head-pair flash attention, software-pipelined ----
            blocks = []
            group_start = []
            fi = 0
            for qj in range(4):
                group_start.append(fi)
                for hp in range(2):
                    n_kc = (qj + 1) * 4
                    order = list(range(qj * 4, n_kc)) + list(range(0, qj * 4))
                    blocks.append((qj, hp, order))
                    fi += len(order)
            flat = [(bi, j) for bi, (_, _, order) in enumerate(blocks)
                    for j in range(len(order))]
            # Later groups' QKV/V chains AND the previous group's proj
            # chains are spread one-at-a-time through the attention stream:
            # the PE absorbs them between S/PV pairs (keeping its HAM clock
            # warm) while the scalar engine grinds exps. QKV chains for
            # group g must be emitted before the S-emitter (LOOKAHEAD
            # ahead) reaches group_start[g].
            inject = {
                0: [(emit_qk_chain, (1, 0)), (emit_qk_chain, (1, 1))],
                1: [(emit_qk_chain, (1, 2)), (emit_qk_chain, (1, 3))],
                2: [(emit_v_chain, (4,)), (emit_v_chain, (5,))],
                3: [(emit_v_chain, (6,)), (emit_v_chain, (7,))],
                # group 1 (flat 8..23) hosts QKV/V(2)
                8: [(emit_qk_chain, (2, 0))],
                10: [(emit_qk_chain, (2, 1))],
                12: [(emit_qk_chain, (2, 2))],
                14: [(emit_qk_chain, (2, 3))],
                16: [(emit_v_chain, (8,))],
                18: [(emit_v_chain, (9,))],
                20: [(emit_v_chain, (10,))],
                21: [(emit_v_chain, (11,))],
                # group 2 (flat 24..47) hosts QKV/V(3) + proj(0)
                25: [(emit_proj_chain, (0, 0))],
                26: [(emit_qk_chain, (3, 0))],
                28: [(emit_proj_chain, (0, 1))],
                29: [(emit_qk_chain, (3, 1))],
                31: [(emit_proj_chain, (1, 0))],
                32: [(emit_qk_chain, (3, 2))],
                34: [(emit_proj_chain, (1, 1))],
                35: [(emit_qk_chain, (3, 3))],
                37: [(emit_proj_chain, (2, 0))],
                38: [(emit_v_chain, (12,))],
                39: [(emit_proj_chain, (2, 1))],
                40: [(emit_v_chain, (13,))],
                42: [(emit_v_chain, (14,))],
                43: [(emit_proj_chain, (3, 0))],
                44: [(emit_v_chain, (15,))],
                46: [(emit_proj_chain, (3, 1))],
                # group 3 (flat 48..79, ACT-heavy) hosts proj(1) + proj(2)
                49: [(emit_proj_chain, (4, 0))],
                51: [(emit_proj_chain, (4, 1))],
                53: [(emit_proj_chain, (5, 0))],
                55: [(emit_proj_chain, (5, 1))],
                57: [(emit_proj_chain, (6, 0))],
                59: [(emit_proj_chain, (6, 1))],
                61: [(emit_proj_chain, (7, 0))],
                63: [(emit_proj_chain, (7, 1))],
                64: [(emit_proj_chain, (8, 0))],
                66: [(emit_proj_chain, (8, 1))],
                68: [(emit_proj_chain, (9, 0))],
                70: [(emit_proj_chain, (9, 1))],
                72: [(emit_proj_chain, (10, 0))],
                74: [(emit_proj_chain, (10, 1))],
                76: [(emit_proj_chain, (11, 0))],
                78: [(emit_proj_chain, (11, 1))],
            }
            # Deep lookahead: exps pre-run up to 6 pairs ahead (pt bufs=12
            # bounds the exp->PV lag), so the scalar engine banks future
            # groups' exp work during the PE-bound chain-heavy stretches
            LOOKAHEAD = 6
            pv_tiles = {}
            pts = {}

            # Upfront chains: block (0,0) only needs the m=0/m=2 chains, so
            # emit those first and seed the S pipeline immediately — the
            # scalar engine starts on exps ~5us earlier.
            emit_qk_chain(0, 0)
            emit_qk_chain(0, 2)

            def emit_s(idx):
                bi, j = flat[idx]
                qj, hp, order = blocks[bi]
                kc = order[j]
                q0 = qj * CQ
                qoff = max(0, kc * CK - q0)   # causal trim: 0/128/256/384
                width = CQ - qoff
                sp = psum.tile([128, 2 * CQ], f32, tag="sp", bufs=2,
                               name="sp")
                # the two heads use disjoint PE row groups -> concurrent
                nc.tensor.matmul(
                    sp[:, 0:width],
                    kt_sb[hp][kc // 4][0:D, (kc % 4) * 128:(kc % 4 + 1) * 128],
                    qt_sb[hp][qj][0:D, qoff:CQ],
                    start=True, stop=True)
                nc.tensor.matmul(
                    sp[:, CQ:CQ + width],
                    kt_sb[hp][kc // 4][D:128, (kc % 4) * 128:(kc % 4 + 1) * 128],
                    qt_sb[hp][qj][D:128, qoff:CQ],
                    start=True, stop=True)
                pt = wpool.tile([128, 2 * CQ], DT, tag="pT", bufs=12)
                sview = sp.rearrange("p (two q) -> p two q", two=2)[:, :, 0:width]
                pview = pt.rearrange("p (two q) -> p two q", two=2)[:, :, 0:width]
                nc.scalar.activation(pview, sview, Exp, scale=1.0 / 8.0)
                if kc >= qj * 4:   # diagonal chunk: mask q < k on both heads
                    nc.gpsimd.affine_select(
                        out=pview, in_=pview,
                        compare_op=mybir.AluOpType.is_ge,
                        fill=0.0, base=0,
                        pattern=[[0, 2], [1, width]], channel_multiplier=-1)
                pts[(bi, kc)] = (pt, qoff, width)

            for idx in range(4):
                emit_s(idx)
            emit_qk_chain(0, 1)
            emit_qk_chain(0, 3)
            for idx in range(4, LOOKAHEAD):
                emit_s(idx)
            for tt in range(4):
                emit_v_chain(tt)
            for i, (bi, j) in enumerate(flat):
                qj, hp, order = blocks[bi]
                kc = order[j]
                n_kc = len(order)
                q0 = qj * CQ
                if j == 0:
                    pv_tiles[bi] = psum.tile([D + 1, 2 * CQ], f32, tag="pv",
                                             bufs=1, name="pv")
                pv = pv_tiles[bi]
                pt, qoff, width = pts.pop((bi, kc))
                nc.tensor.matmul(
                    pv[:, qoff:qoff + width],
                    v_sb[kc][:, (2 * hp) * (D + 1):(2 * hp + 1) * (D + 1)],
                    pt[:, 0:width],
                    start=(j == 0), stop=(j == n_kc - 1))
                nc.tensor.matmul(
                    pv[:, CQ + qoff:CQ + qoff + width],
                    v_sb[kc][:, (2 * hp + 1) * (D + 1):(2 * hp + 2) * (D + 1)],
                    pt[:, CQ:CQ + width],
                    start=(j == 0), stop=(j == n_kc - 1))
                for fn, args in inject.get(i, ()):
                    fn(*args)
                if i + LOOKAHEAD < len(flat):
                    emit_s(i + LOOKAHEAD)
                if j != n_kc - 1:
                    continue
                # block complete: evacuate PV fast (pvs on DVE, the ones-row
                # on the idle-ish scalar engine) so the single pv PSUM slot
                # unlocks for the next block, then recip/broadcast/scale.
                del pv_tiles[bi]
                if qj == 3:
                    # group 3 is ACT-bound: ones-row copy stays off the
                    # scalar engine, and goes FIRST on the DVE queue so the
                    # recip chain (the exposed tail path) starts earlier
                    ls = wpool.tile([1, 2 * CQ], f32, tag="ls", bufs=2)
                    nc.vector.tensor_copy(ls[:], pv[D:D + 1, :])
                    pvs = wpool.tile([D, 2 * CQ], f32, tag="pvs", bufs=2)
                    nc.vector.tensor_copy(pvs[:], pv[0:D, :])
                else:
                    pvs = wpool.tile([D, 2 * CQ], f32, tag="pvs", bufs=2)
                    nc.vector.tensor_copy(pvs[:], pv[0:D, :])
                    ls = wpool.tile([1, 2 * CQ], f32, tag="ls", bufs=2)
                    nc.scalar.copy(ls[:], pv[D:D + 1, :])
                r = wpool.tile([1, 2 * CQ], f32, tag="r", bufs=2)
                with nc.allow_low_precision(reason="softmax denom"):
                    # approx_fast needs SBUF input at partition base 0
                    nc.vector.reciprocal_approx_fast(r[:], ls[:])
                rbs = wpool.tile([D, 2 * CQ], f32, tag="rbs", bufs=2)
                nc.gpsimd.partition_broadcast(rbs[:], r[:])
                nc.vector.tensor_mul(
                    ao_sb[hp][0:D, q0:q0 + CQ], pvs[:, 0:CQ], rbs[:, 0:CQ])
                nc.vector.tensor_mul(
                    ao_sb[hp][D:128, q0:q0 + CQ], pvs[:, CQ:2 * CQ],
                    rbs[:, CQ:2 * CQ])
                if hp == 1 and qj == 3:
                    # last block: keep the PE's HAM clock warm with dummy
                    # matmuls while its finalize chain drains, then proj.
                    sp_w = psum.tile([128, 2 * CQ], f32, tag="sp", bufs=2,
                                     name="spw")
                    tailw["ps"] = sp_w
                    for k in range(20):
                        nc.tensor.matmul(sp_w[:, 0:CQ], wmm[:, 0:128],
                                         wmm[:],
                                         start=(k == 0), stop=(k == 19))
                    for tt in range(12, 16):
                        emit_proj_chain(tt, 0)
                        emit_proj_chain(tt, 1)

    nc.compile()
    return nc


def _get_compiled():
    global _COMPILED
    if _COMPILED is None:
        _COMPILED = _build()
    return _COMPILED


def make_in_maps(x, Wq, bq, Wk, Wv, Wp):
    in_maps = []
    for c in range(N_CORES):
        b, g = divmod(c, 4)
        sl = slice(g * OL, (g + 1) * OL)
        XT = np.ascontiguousarray(x[b].T)                      # [C, T]
        AB = (XT.reshape(KC, 128, 4, CQ).transpose(1, 2, 0, 3)
              .reshape(128, 4 * KC * CQ))
        WQK = np.concatenate([Wq[sl].T, Wk[sl].T], axis=1)     # [C, 512]
        WB = WQK.reshape(KC, 128, CQ).transpose(1, 0, 2).reshape(128, KC * CQ)
        WVB = (Wv[sl].T.reshape(KC, 128, OL).transpose(1, 0, 2)
               .reshape(128, KC * OL))
        WPB = (Wp[:, sl].T.reshape(2, 128, C).transpose(1, 0, 2)
               .reshape(128, 2 * C))
        in_maps.append({
            "AB": np.ascontiguousarray(AB).astype(NP_DT),
            "WB": np.ascontiguousarray(WB).astype(NP_DT),
            "WVB": np.ascontiguousarray(WVB).astype(NP_DT),
            "WPB": np.ascontiguousarray(WPB).astype(NP_DT),
            "BQ": np.ascontiguousarray(bq[sl].reshape(2, 128).T).astype(
                np.float32),
        })
    return in_maps


_RUNNER = None


def _make_runner():
    """Build the 8-core shard_map executable once (run_bass_via_pjrt re-jits
    on every call; this caches the traced/compiled callable)."""
    import jax
    from jax.sharding import Mesh, PartitionSpec
    from jax.experimental.shard_map import shard_map
    import concourse.mybir as mybir_
    from concourse import bass2jax

    nc = _get_compiled()
    bass2jax.install_neuronx_cc_hook()

    partition_name = (nc.partition_id_tensor.name
                      if nc.partition_id_tensor else None)
    in_names, out_names, out_avals, zero_outs = [], [], [], []
    for alloc in nc.m.functions[0].allocations:
        if not isinstance(alloc, mybir_.MemoryLocationSet):
            continue
        name = alloc.memorylocations[0].name
        if alloc.kind == "ExternalInput":
            if name != partition_name:
                in_names.append(name)
        elif alloc.kind == "ExternalOutput":
            shape = tuple(alloc.tensor_shape)
            dtype = mybir_.dt.np(alloc.dtype)
            out_names.append(name)
            out_avals.append(jax.core.ShapedArray(shape, dtype))
            zero_outs.append(np.zeros(shape, dtype))
    n_params = len(in_names)
    n_outs = len(out_avals)
    all_in_names = list(in_names) + list(out_names)
    if partition_name is not None:
        all_in_names.append(partition_name)
    donate = tuple(range(n_params, n_params + n_outs))

    def _body(*args):
        operands = list(args)
        if partition_name is not None:
            operands.append(bass2jax.partition_id_tensor())
        outs = bass2jax._bass_exec_p.bind(
            *operands,
            out_avals=tuple(out_avals),
            in_names=tuple(all_in_names),
            out_names=tuple(out_names),
            lowering_input_output_aliases=(),
            sim_require_finite=True,
            sim_require_nnan=True,
            nc=nc,
        )
        return tuple(outs)

    devices = jax.devices()[:N_CORES]
    mesh = Mesh(np.asarray(devices), ("core",))
    in_specs = (PartitionSpec("core"),) * (n_params + n_outs)
    out_specs = (PartitionSpec("core"),) * n_outs
    sharded = jax.jit(
        shard_map(_body, mesh=mesh, in_specs=in_specs, out_specs=out_specs,
                  check_rep=False),
        donate_argnums=donate, keep_unused=True)

    def run(in_maps):
        per_core = [[np.asarray(m[name]) for name in in_names]
                    for m in in_maps]
        concat_in = [
            np.concatenate([per_core[c][i] for c in range(N_CORES)], axis=0)
            for i in range(n_params)]
        concat_zeros = [
            np.zeros((N_CORES * z.shape[0], *z.shape[1:]), z.dtype)
            for z in zero_outs]
        out_arrs = sharded(*concat_in, *concat_zeros)
        return [
            {name: np.asarray(out_arrs[i]).reshape(
                N_CORES, *out_avals[i].shape)[c]
             for i, name in enumerate(out_names)}
            for c in range(N_CORES)]

    return run


def _get_runner():
    global _RUNNER
    if _RUNNER is None:
        _RUNNER = _make_runner()
    return _RUNNER


def _axon_reset():
    try:
        import ctypes
        lib = ctypes.CDLL("/opt/axon/libaxon_pjrt.so")
        if hasattr(lib, "axon_reset"):
            lib.axon_reset()
    except Exception:
        pass


def kernel(x, Wq, bq, Wk, bk, Wv, bv, Wp, bp):
    x = np.asarray(x, dtype=np.float32)
    Wq = np.asarray(Wq, dtype=np.float32)
    bq = np.asarray(bq, dtype=np.float32)
    Wk = np.asarray(Wk, dtype=np.float32)
    Wv = np.asarray(Wv, dtype=np.float32)
    Wp = np.asarray(Wp, dtype=np.float32)
    bv = np.asarray(bv, dtype=np.float32)
    bp = np.asarray(bp, dtype=np.float32)

    in_maps = make_in_maps(x, Wq, bq, Wk, Wv, Wp)

    results = None
    for attempt in range(3):
        try:
            results = _get_runner()(in_maps)
            break
        except Exception:
            if attempt == 2:
                raise
            _axon_reset()  # recover a wedged accelerator and retry

    extra = bv @ Wp.T + bp  # bv/bp fold out of the device kernel
    out = np.empty((B, T, C), dtype=np.float32)
    for b in range(B):
        acc = results[4 * b]["Y"].astype(np.float32)
        for g in range(1, 4):
            acc = acc + results[4 * b + g]["Y"].astype(np.float32)
        out[b] = acc + extra
    return out
